# revision 24
# baseline (speedup 1.0000x reference)
"""Trainium2 Bass kernel for a dense transformer block (B=2,S=2048,E=768,H=12,D=64,F=3072).

Sharding: 8 cores = 2 batch groups x 4 cores. Within a batch group each core
computes attention for 3 of the 12 heads over the full sequence, partial output
projections are combined with a 4-core fp16 ReduceScatter, and each core then
runs the FFN on its 512 rows.

Key structure choices:
- Masked keys are zeroed in V (per-token 0/1 multiply) instead of per-key exp
  biases, so every exp uses one constant bias and adjacent score chunks share
  a single [128,1024] activation (2 PSUM banks).
- Causal masking is done by restricting matmul/exp column ranges per t-chunk;
  only the single 128x128 diagonal block needs an elementwise 0/1 multiply.
- All-masked rows get their diagonal value injected via one extra matmul
  (v_raw^T @ e^-8*diag(bad)) inside the AV accumulation group.
- Softmax reciprocals run as exp(-ln(d)) on the scalar engine over a [3,512]
  tile per row-block (the DVE iterative divide is ~4us per 512-elem row).
- rsqrt in layernorm = exp(-0.5*ln(var+eps)): keeps the scalar engine on one
  activation table set (no exp<->sqrt table thrashing).
- Wo projection and the middle FFN matmul (h1@W2) run in fp8e4m3 DoubleRow
  (2 contraction rows/cycle); h1 is written in fp8 by the ReLU activation.
- FFN1 is split into column groups [128:512] and [0:128] so the bulk of it
  overlaps the last ReduceScatter (row blocks are processed J=3,2,1,0).
"""

import sys

if "/opt/trn_rl_repo" not in sys.path:
    sys.path.insert(0, "/opt/trn_rl_repo")

import numpy as np
import ml_dtypes

import concourse.bacc as bacc
import concourse.mybir as mybir
import concourse.tile as tile
from concourse.bass_utils import run_bass_kernel_spmd

B, S, E, H, D, F = 2, 2048, 768, 12, 64, 3072
NCORES = 8
R = 4          # cores per batch group
HPC = 3        # heads per core
MYR = S // R   # rows per core after reduce-scatter (512)
EC = E // 128  # 6 e-chunks
SC = S // 128  # 16 s-chunks of 128
FC = F // 128  # 24 f-chunks
VW = 256       # padded V width (3 heads x 65 = 195 -> 256)

f32 = mybir.dt.float32
f16 = mybir.dt.float16
f8 = mybir.dt.float8e4
AF = mybir.ActivationFunctionType
ALU = mybir.AluOpType
PM = mybir.MatmulPerfMode

E4NP = ml_dtypes.float8_e4m3
LN16 = float(np.log(16.0))

_CACHE = {}


def _declare_io(nc, has_inj):
    t = {}
    F16_IN = {"wq", "wv", "m_tri", "inj", "ident", "sel", "w1", "w3", "woq"}
    F8_IN = {"w2q"}

    def inp(name, shape):
        dt = f16 if name in F16_IN else (f8 if name in F8_IN else f32)
        t[name] = nc.dram_tensor(name, list(shape), dt, kind="ExternalInput").ap()

    inp("ident", (128, 128))
    inp("m_tri", (128, 128))
    if has_inj:
        inp("inj", (128, 128))
    inp("sel", (65, 192))
    inp("bqc", (128, 2))
    inp("shiftb", (128, 1))
    inp("bv_bc", (128, VW))
    inp("maskc", (128, SC))
    inp("b1c8", (128, FC))
    inp("b2c", (128, FC))
    inp("b3_bc", (128, E))
    inp("xb", (128, SC * E))
    inp("xmy", (128, 4 * E))          # my rows of x[b] with bo pre-added
    inp("wq", (128, EC * 192))
    inp("wv", (128, EC * VW))
    inp("woq", (128, 2 * E))          # fp16: plane0 heads01, plane1 head2+pad
    inp("w1", (128, EC * F))
    inp("w2q", (128, FC * F))         # fp8 x16
    inp("w3", (128, FC * E))
    t["out"] = nc.dram_tensor("out", [128, 4 * E], f32, kind="ExternalOutput").ap()
    return t


def _ln_chunk(nc, pool, x_chunk, out_chunk, out_scale=1.0,
              apply_engine="scalar"):
    """LN a [128, ncols] fp32 chunk into out_chunk, eps=1e-5.

    out = (x - mu) * rsqrt(var+eps) * out_scale, rsqrt via exp(-0.5*ln).
    """
    stats = pool.tile([128, 12], f32, tag="ln_stats")
    nc.vector.bn_stats(stats[:, 0:6], x_chunk[:, 0:384])
    nc.vector.bn_stats(stats[:, 6:12], x_chunk[:, 384:768])
    mv = pool.tile([128, 2], f32, tag="ln_mv")
    nc.vector.bn_aggr(mv[:], stats[:])
    veps = pool.tile([128, 1], f32, tag="ln_veps")
    nc.vector.tensor_scalar_add(veps[:], mv[:, 1:2], 1e-5)
    lnv = pool.tile([128, 1], f32, tag="ln_lnv")
    nc.scalar.activation(lnv[:], veps[:], AF.Ln, bias=0.0, scale=1.0)
    rsig = pool.tile([128, 1], f32, tag="ln_rsig")
    bias = float(np.log(out_scale)) if out_scale != 1.0 else 0.0
    nc.scalar.activation(rsig[:], lnv[:], AF.Exp, bias=bias, scale=-0.5)
    negmurs = pool.tile([128, 1], f32, tag="ln_negmurs")
    nc.vector.scalar_tensor_tensor(
        negmurs[:], mv[:, 0:1], -1.0, rsig[:], ALU.mult, ALU.mult
    )
    if apply_engine == "vector":
        # out = (x + negmurs) * rsig via two per-partition scalars
        nc.vector.tensor_scalar(out_chunk, x_chunk, negmurs[:], rsig[:],
                                ALU.add, ALU.mult)
    else:
        nc.scalar.activation(out_chunk, x_chunk, AF.Identity,
                             bias=negmurs[:], scale=rsig[:])


def _build_body(tc, t, has_inj):
    nc = tc.nc

    with tc.tile_pool(name="constp", bufs=1) as constp, \
         tc.tile_pool(name="lnstat", bufs=4) as lnstat, \
         tc.tile_pool(name="dramp", bufs=1, space="DRAM") as dramp, \
         tc.tile_pool(name="w1pool", bufs=1) as w1pool, \
         tc.tile_pool(name="yp", bufs=1) as yp:
        proj_J = [dramp.tile([MYR, E], f16, name=f"projb_{J}") for J in range(4)]
        rs_J = [dramp.tile([128, E], f16, name=f"rsout_{J}") for J in range(4)]
        warm_in = dramp.tile([4, 64], f16, name="warm_in")
        warm_out = dramp.tile([1, 64], f16, name="warm_out")

        # small constants first so transposes never wait behind x
        ident = constp.tile([128, 128], f16)
        nc.sync.dma_start(ident[:], t["ident"][:])
        warm_sb = constp.tile([4, 64], f16)
        nc.vector.memset(warm_sb[:], 0.0)
        nc.sync.dma_start(warm_in[:, :], warm_sb[:])
        nc.gpsimd.collective_compute(
            "ReduceScatter", ALU.add,
            replica_groups=[[0, 1, 2, 3], [4, 5, 6, 7]],
            ins=[warm_in[:, :].opt()], outs=[warm_out[:, :].opt()],
        )
        m_tri = constp.tile([128, 128], f16)
        nc.sync.dma_start(m_tri[:], t["m_tri"][:])
        if has_inj:
            inj = constp.tile([128, 128], f16)
            nc.sync.dma_start(inj[:], t["inj"][:])
        sel = constp.tile([65, 192], f16)
        nc.sync.dma_start(sel[:], t["sel"][:])
        bqc = constp.tile([128, 2], f32)
        nc.sync.dma_start(bqc[:], t["bqc"][:])
        bv_bc = constp.tile([128, VW], f32)
        nc.sync.dma_start(bv_bc[:], t["bv_bc"][:])
        maskc = constp.tile([128, SC], f32)
        nc.sync.dma_start(maskc[:], t["maskc"][:])
        b1c8 = constp.tile([128, FC], f32)
        nc.sync.dma_start(b1c8[:], t["b1c8"][:])
        b2c = constp.tile([128, FC], f32)
        nc.sync.dma_start(b2c[:], t["b2c"][:])
        b3_bc = constp.tile([128, E], f32)
        nc.sync.dma_start(b3_bc[:], t["b3_bc"][:])
        bm8 = constp.tile([128, 1], f32)
        nc.sync.dma_start(bm8[:], t["shiftb"][:])
        bl16 = constp.tile([128, 1], f32)
        nc.vector.memset(bl16[:], LN16)

        # long-lived FFN-input tiles
        y1 = yp.tile([128, 4 * E], f32)
        y13 = y1[:].rearrange("p (c e) -> p c e", c=4)
        ylnT = yp.tile([128, EC * MYR], f16)
        ylnT3 = ylnT[:].rearrange("p (e s) -> p e s", e=EC)
        xmy_sb = yp.tile([128, 4 * E], f32)
        xmy3 = xmy_sb[:].rearrange("p (c e) -> p c e", c=4)
        h1T = yp.tile([128, FC * MYR], f8)
        h13 = h1T[:].rearrange("p (f s) -> p f s", f=FC)

        with tc.tile_pool(name="aconstp", bufs=1) as aconstp:
            with tc.tile_pool(name="qvp", bufs=1) as qvp:
                # ------------ Phase A: LN1(x_b) + transpose -> ln1T --------
                with tc.tile_pool(name="ln1Tp", bufs=1) as ln1Tp:
                    ln1T = ln1Tp.tile([128, EC * S], f16)
                    ln1T3 = ln1T[:].rearrange("p (e s) -> p e s", e=EC)
                    with tc.tile_pool(name="xp", bufs=1) as xp, \
                         tc.tile_pool(name="lnxp", bufs=3) as lnxp, \
                         tc.tile_pool(name="psA", bufs=4, space="PSUM") as psA:
                        x_sb = xp.tile([128, SC * E], f32)
                        x3 = x_sb[:].rearrange("p (s e) -> p s e", s=SC)
                        xb3 = t["xb"][:].rearrange("p (s e) -> p s e", s=SC)
                        for xc in range(4):
                            nc.sync.dma_start(
                                x3[:, 4 * xc:4 * (xc + 1), :],
                                xb3[:, 4 * xc:4 * (xc + 1), :],
                            )
                        # queue the rest of the inputs behind x
                        nc.sync.dma_start(xmy_sb[:], t["xmy"][:])
                        wq = aconstp.tile([128, EC * 192], f16)
                        nc.sync.dma_start(wq[:], t["wq"][:])
                        wv = aconstp.tile([128, EC * VW], f16)
                        nc.sync.dma_start(wv[:], t["wv"][:])
                        woq = aconstp.tile([128, 2 * E], f16)
                        nc.sync.dma_start(woq[:], t["woq"][:])
                        w1_sb = w1pool.tile([128, EC * F], f16)
                        nc.sync.dma_start(
                            w1_sb[:].rearrange("p (e f) -> p e f", e=EC),
                            t["w1"][:].rearrange("p (e f) -> p e f", e=EC),
                        )
                        for sc in range(SC):
                            lnx = lnxp.tile([128, E], f16, tag="lnx")
                            _ln_chunk(nc, lnstat, x3[:, sc, :], lnx[:],
                                      apply_engine=("vector" if sc % 2 else
                                                    "scalar"))
                            pt = psA.tile([128, EC * 128], f16, tag="pt")
                            for ec in range(EC):
                                nc.tensor.transpose(
                                    pt[:, ec * 128:(ec + 1) * 128],
                                    lnx[:, ec * 128:(ec + 1) * 128], ident[:]
                                )
                            dst = ln1T3[:, :, sc * 128:(sc + 1) * 128]
                            if sc % 2 == 0:
                                nc.vector.tensor_copy(dst, pt[:])
                            else:
                                nc.scalar.copy(dst, pt[:])

                    # ------------ Phase B: Q^T and V projections ----------
                    wq3 = wq[:].rearrange("p (e m) -> p e m", e=EC)
                    wv3 = wv[:].rearrange("p (e m) -> p e m", e=EC)
                    qh = [qvp.tile([128, S], f16, name=f"qh_{i}")
                          for i in range(HPC)]
                    v_sb = qvp.tile([128, SC * VW], f16)
                    v3 = v_sb[:].rearrange("p (s v) -> p s v", s=SC)
                    v0_raw = qvp.tile([128, VW], f16)
                    for i in range(HPC):
                        nc.vector.memset(qh[i][64:128, :], 0.0)
                    with tc.tile_pool(name="psQ", bufs=3, space="PSUM") as psQ:
                        for g in range(2):
                            m = 128 if g == 0 else 64
                            for s4 in range(4):
                                pq = psQ.tile([128, 512], f32, tag="pq")
                                for ec in range(EC):
                                    nc.tensor.matmul(
                                        pq[:m, :],
                                        wq3[:, ec, g * 128:g * 128 + m],
                                        ln1T3[:, ec, s4 * 512:(s4 + 1) * 512],
                                        start=(ec == 0), stop=(ec == EC - 1),
                                    )
                                if g == 0:
                                    nc.vector.tensor_scalar_add(
                                        qh[0][0:64, s4 * 512:(s4 + 1) * 512],
                                        pq[0:64, :], bqc[0:64, 0:1],
                                    )
                                    nc.vector.tensor_scalar_add(
                                        qh[1][0:64, s4 * 512:(s4 + 1) * 512],
                                        pq[64:128, :], bqc[64:128, 0:1],
                                    )
                                else:
                                    nc.vector.tensor_scalar_add(
                                        qh[2][0:64, s4 * 512:(s4 + 1) * 512],
                                        pq[0:64, :], bqc[0:64, 1:2],
                                    )
                        for sc in range(SC):
                            pv = psQ.tile([128, 512], f32, tag="pq")
                            for ec in range(EC):
                                nc.tensor.matmul(
                                    pv[:, :VW],
                                    ln1T3[:, ec, sc * 128:(sc + 1) * 128],
                                    wv3[:, ec, :],
                                    start=(ec == 0), stop=(ec == EC - 1),
                                )
                            if sc == 0:
                                nc.vector.tensor_tensor(
                                    v0_raw[:], pv[:, :VW], bv_bc[:], ALU.add
                                )
                                nc.vector.tensor_scalar_mul(
                                    v3[:, 0, :], v0_raw[:], maskc[:, 0:1]
                                )
                            else:
                                nc.vector.tensor_tensor(
                                    v3[:, sc, :], pv[:, :VW], bv_bc[:], ALU.add
                                )
                                nc.vector.tensor_scalar_mul(
                                    v3[:, sc, :], v3[:, sc, :], maskc[:, sc:sc + 1]
                                )

                # -------- Phase C: attention + Wo + chunked RS -------------
                oTq = qvp.tile([128, 2 * S], f16, name="oTq")
                oTq3 = oTq[:].rearrange("p (k s) -> p k s", k=2)
                nc.vector.memset(oTq3[64:128, 1, :], 0.0)
                woq3 = woq[:].rearrange("p (k e) -> p k e", k=2)
                with tc.tile_pool(name="pexp", bufs=4) as pexp, \
                     tc.tile_pool(name="osml", bufs=8) as osml, \
                     tc.tile_pool(name="projp", bufs=4) as projp, \
                     tc.tile_pool(name="lnyp", bufs=2) as lnyp, \
                     tc.tile_pool(name="psP", bufs=2, space="PSUM") as psP, \
                     tc.tile_pool(name="psO", bufs=2, space="PSUM") as psO, \
                     tc.tile_pool(name="psM", bufs=1, space="PSUM") as psM, \
                     tc.tile_pool(name="psE", bufs=1, space="PSUM") as psE:
                    def make_steps(J, hh, po, av_state):
                        """Pair steps for one head: (emit_scores, emit_avs)."""
                        qT = qh[hh]
                        steps = []
                        for p2 in range(2 * J):
                            def mk_off(p2=p2, hh=hh):
                                box = {}
                                def scores():
                                    pp2 = psP.tile([128, 1024], f32, tag="pp2")
                                    for half in (0, 1):
                                        tci = 2 * p2 + half
                                        nc.tensor.matmul(
                                            pp2[:, half * 512:(half + 1) * 512],
                                            qT[:, tci * 128:(tci + 1) * 128],
                                            qT[:, J * 512:(J + 1) * 512],
                                            start=True, stop=True,
                                        )
                                    pe2 = pexp.tile([128, 1024], f16, tag="pe2")
                                    nc.scalar.activation(pe2[:], pp2[:], AF.Exp,
                                                         bias=bm8[:], scale=0.125)
                                    box["pe2"] = pe2
                                def avs():
                                    pe2 = box["pe2"]
                                    for half in (0, 1):
                                        tci = 2 * p2 + half
                                        _emit_av(po, hh,
                                                 v3[:, tci, 65 * hh:65 * hh + 65],
                                                 pe2[:, half * 512:(half + 1) * 512],
                                                 0, av_state)
                                return scores, avs
                            steps.append(mk_off())
                        for dp in range(2):
                            def mk_diag(dp=dp, hh=hh):
                                k0 = 2 * dp
                                box = {}
                                def scores():
                                    ppd = psP.tile([128, 1024], f32, tag="pp2")
                                    ppd3 = ppd[:].rearrange("p (k s) -> p k s", k=2)
                                    for i in (0, 1):
                                        tci = 4 * J + k0 + i
                                        nc.tensor.matmul(
                                            ppd3[:, i, 128 * k0:512],
                                            qT[:, tci * 128:(tci + 1) * 128],
                                            qT[:, J * 512 + 128 * k0:(J + 1) * 512],
                                            start=True, stop=True,
                                        )
                                    ped = pexp.tile([128, 1024], f16, tag="pe2")
                                    ped3 = ped[:].rearrange("p (k s) -> p k s", k=2)
                                    nc.scalar.activation(
                                        ped3[:, :, 128 * k0:512],
                                        ppd3[:, :, 128 * k0:512],
                                        AF.Exp, bias=bm8[:], scale=0.125,
                                    )
                                    box["ped3"] = ped3
                                def avs():
                                    ped3 = box["ped3"]
                                    for i in (0, 1):
                                        k = k0 + i
                                        tci = 4 * J + k
                                        nc.vector.tensor_mul(
                                            ped3[:, i, 128 * k:128 * (k + 1)],
                                            ped3[:, i, 128 * k:128 * (k + 1)],
                                            m_tri[:],
                                        )
                                        _emit_av(po, hh,
                                                 v3[:, tci, 65 * hh:65 * hh + 65],
                                                 ped3[:, i, 128 * k:512],
                                                 128 * k, av_state)
                                return scores, avs
                            steps.append(mk_diag())
                        return steps

                    def _emit_av(po, hh, vl, pr, lo, st):
                        st["n"] += 1
                        is_last = st["n"] == st["total"]
                        nc.tensor.matmul(
                            po[:65, lo:512], vl, pr,
                            start=(st["n"] == 1),
                            stop=(is_last and not st["inj"]),
                            skip_group_check=True,
                        )
                        if is_last and st["inj"]:
                            nc.tensor.matmul(
                                po[:65, 0:128],
                                v0_raw[:, 65 * hh:65 * hh + 65],
                                inj[:],
                                start=False, stop=True,
                                skip_group_check=True,
                            )

                    def drain_head(po, hh, dn, sb_os):
                        sb_o = osml.tile([64, 512], f16, tag="sb_o")
                        nc.vector.tensor_copy(sb_o[:], po[0:64, :])
                        nc.vector.tensor_copy(
                            dn[32 * hh:32 * hh + 1, :], po[64:65, :]
                        )
                        sb_os.append(sb_o)

                    def make_recip(J, dn):
                        def emit():
                            lnd = osml.tile([65, 512], f32, tag="lnd")
                            nc.scalar.activation(lnd[:], dn[:], AF.Ln,
                                                 bias=0.0, scale=1.0)
                            rcp = osml.tile([65, 512], f16, tag="rcp")
                            nc.scalar.activation(rcp[:], lnd[:], AF.Exp,
                                                 bias=0.0, scale=-1.0)
                            return rcp
                        return emit

                    def make_rest(J, sb_os):
                        def emit(rcp):
                            for hh in range(HPC):
                                pb = psM.tile([128, 512], f32, tag="misc")
                                nc.tensor.matmul(
                                    pb[0:64, :], sel[:, 64 * hh:64 * (hh + 1)],
                                    rcp[:], start=True, stop=True,
                                )
                                sb_b = osml.tile([64, 512], f16, tag="sb_b")
                                nc.vector.tensor_copy(sb_b[:], pb[0:64, :])
                                plane, off = (0, 64 * hh) if hh < 2 else (1, 0)
                                nc.vector.tensor_mul(
                                    oTq3[off:off + 64, plane,
                                         J * 512:(J + 1) * 512],
                                    sb_os[hh][:], sb_b[:],
                                )
                            for sl in range(4):
                                st_ = 4 * J + sl
                                for half in (0, 1):
                                    pw = psM.tile([128, 512], f32, tag="misc")
                                    nc.tensor.matmul(
                                        pw[:, 0:384],
                                        oTq3[:, 0, st_ * 128:(st_ + 1) * 128],
                                        woq3[:, 0, half * 384:(half + 1) * 384],
                                        start=True, stop=False,
                                    )
                                    nc.tensor.matmul(
                                        pw[:, 0:384],
                                        oTq3[0:64, 1, st_ * 128:(st_ + 1) * 128],
                                        woq3[0:64, 1, half * 384:(half + 1) * 384],
                                        start=False, stop=True,
                                    )
                                    prj = projp.tile([128, 384], f16, tag="prj")
                                    if half == 0:
                                        nc.vector.tensor_copy(prj[:], pw[:, 0:384])
                                    else:
                                        nc.scalar.copy(prj[:], pw[:, 0:384])
                                    nc.sync.dma_start(
                                        proj_J[J][sl * 128:(sl + 1) * 128,
                                                  half * 384:(half + 1) * 384],
                                        prj[:],
                                    )
                            nc.gpsimd.collective_compute(
                                "ReduceScatter",
                                ALU.add,
                                replica_groups=[[0, 1, 2, 3], [4, 5, 6, 7]],
                                ins=[proj_J[J][:, :].opt()],
                                outs=[rs_J[J][:, :].opt()],
                            )
                            rs_sb = yp.tile([128, E], f16, tag="rs_sb",
                                            name=f"rs_sb_{J}", bufs=4)
                            nc.gpsimd.dma_start(rs_sb[:], rs_J[J][:, :])
                            return rs_sb
                        return emit

                    def make_post(J, rs_sb):
                        def emit():
                            nc.vector.tensor_add(y13[:, J, :], rs_sb[:],
                                                 xmy3[:, J, :])
                            lny = lnyp.tile([128, E], f16, tag="lny")
                            _ln_chunk(nc, lnstat, y13[:, J, :], lny[:])
                            nc.vector.tensor_tensor(
                                y13[:, J, :], y13[:, J, :], b3_bc[:], ALU.add
                            )
                            pt2 = psE.tile([128, EC * 128], f16, tag="pt2")
                            for ec in range(EC):
                                nc.tensor.transpose(
                                    pt2[:, ec * 128:(ec + 1) * 128],
                                    lny[:, ec * 128:(ec + 1) * 128],
                                    ident[:]
                                )
                            nc.scalar.copy(
                                ylnT3[:, :, J * 128:(J + 1) * 128], pt2[:]
                            )
                        return emit

                    pend_recip = None
                    pend_rest = None
                    posts = []
                    for J in (0, 1, 2, 3):
                        # previous block's reciprocal goes first on the
                        # scalar queue so its Wo never stalls on it
                        rcp_prev = pend_recip() if pend_recip else None
                        sb_os = []
                        dn = osml.tile([65, 512], f16, tag="dn",
                                       name=f"dn_{J}", bufs=2)
                        nc.vector.memset(dn[:], 1.0)
                        total = 4 * J + 4
                        st0 = {"n": 0, "total": total,
                               "inj": has_inj and J == 0}
                        st1 = {"n": 0, "total": total,
                               "inj": has_inj and J == 0}
                        st2 = {"n": 0, "total": total,
                               "inj": has_inj and J == 0}
                        po0 = psO.tile([128, 512], f32, tag="po")
                        po1 = psO.tile([128, 512], f32, tag="po")
                        steps0 = make_steps(J, 0, po0, st0)
                        steps1 = make_steps(J, 1, po1, st1)
                        for s0, s1 in zip(steps0, steps1):
                            s0[0]()
                            s1[0]()
                            s0[1]()
                            s1[1]()
                        drain_head(po0, 0, dn, sb_os)
                        drain_head(po1, 1, dn, sb_os)
                        # previous block's divide/Wo/RS while this one runs
                        if pend_rest:
                            posts.append(make_post(J - 1, pend_rest(rcp_prev)))
                        po2 = psO.tile([128, 512], f32, tag="po")
                        for s2 in make_steps(J, 2, po2, st2):
                            s2[0]()
                            s2[1]()
                        drain_head(po2, 2, dn, sb_os)
                        pend_recip = make_recip(J, dn)
                        pend_rest = make_rest(J, sb_os)
                    rcp_last = pend_recip()
                    posts.append(make_post(3, pend_rest(rcp_last)))
                    for p_ in posts[:3]:
                        p_()

                    def ffn1(lo, hi):
                        w = hi - lo
                        for fc in range(FC):
                            pf_t = psP.tile([128, 1024], f32, tag="pp2")
                            pf = pf_t[:, 0:w]
                            for ec in range(EC):
                                nc.tensor.matmul(
                                    pf,
                                    w1_sb[:, (ec * F) + fc * 128:
                                          (ec * F) + (fc + 1) * 128],
                                    ylnT3[:, ec, lo:hi],
                                    start=(ec == 0), stop=(ec == EC - 1),
                                )
                            nc.scalar.activation(
                                h13[:, fc, lo:hi], pf, AF.Relu,
                                bias=b1c8[:, fc:fc + 1], scale=8.0,
                            )

                    ffn1(0, 384)
                    posts[3]()
                    ffn1(384, 512)

        # ---------------- Phase E: FFN on my 512 rows ----------------------
        with tc.tile_pool(name="ffp", bufs=1) as ffp:
            h2T = ffp.tile([128, FC * MYR], f16)
            h23 = h2T[:].rearrange("p (f s) -> p f s", f=FC)

            # FFN2: h2T = relu((1/128)*(W2^T @ h1T) + b2), fp8 DoubleRow
            w23 = t["w2q"][:].rearrange("p (f g) -> p f g", f=FC)
            with tc.tile_pool(name="w2p", bufs=2) as w2p, \
                 tc.tile_pool(name="psF2", bufs=8, space="PSUM") as psF2:
                for f2b in range(6):
                    w2t = w2p.tile([128, FC * 512], f8, tag="w2t")
                    nc.sync.dma_start(
                        w2t[:].rearrange("p (f g) -> p f g", f=FC),
                        w23[:, :, f2b * 512:(f2b + 1) * 512],
                    )
                    w2t3 = w2t[:].rearrange("p (f g) -> p f g", f=FC)
                    ph2 = [
                        psF2.tile([128, 512], f32, tag="ph2",
                                  name=f"ph2_{f2b}_{k}")
                        for k in range(4)
                    ]
                    for i in range(FC // 2):
                        for k in range(4):
                            nc.tensor.matmul(
                                ph2[k][:],
                                w2t3[:, 2 * i:2 * i + 2,
                                     k * 128:(k + 1) * 128],
                                h13[:, 2 * i:2 * i + 2, :],
                                start=(i == 0), stop=(i == FC // 2 - 1),
                                perf_mode=PM.DoubleRow,
                            )
                    for k in range(4):
                        fc2 = f2b * 4 + k
                        nc.scalar.activation(
                            h23[:, fc2, :], ph2[k][:], AF.Relu,
                            bias=b2c[:, fc2:fc2 + 1], scale=1.0 / 128.0,
                        )

            # FFN3: out = h2 @ W3 + b3 + y1 (fp16)
            w33 = t["w3"][:].rearrange("p (f e) -> p f e", f=FC)
            with tc.tile_pool(name="w3p", bufs=3) as w3p, \
                 tc.tile_pool(name="outp", bufs=1) as outp, \
                 tc.tile_pool(name="psF3", bufs=1, space="PSUM") as psF3:
                out_sb = outp.tile([128, 4 * E], f32)
                out3 = out_sb[:].rearrange("p (c e) -> p c e", c=4)
                p3 = [
                    psF3.tile([128, 384], f32, tag=f"p3_{st}_{hf}",
                              name=f"p3_{st}_{hf}")
                    for st in range(4) for hf in range(2)
                ]
                for fc in range(FC):
                    w3t = w3p.tile([128, E], f16, tag="w3t")
                    nc.sync.dma_start(w3t[:], w33[:, fc, :])
                    for st in range(4):
                        for hf in range(2):
                            nc.tensor.matmul(
                                p3[st * 2 + hf][:],
                                h23[:, fc, st * 128:(st + 1) * 128],
                                w3t[:, hf * 384:(hf + 1) * 384],
                                start=(fc == 0), stop=(fc == FC - 1),
                            )
                for st in range(4):
                    for hf in range(2):
                        nc.vector.tensor_add(
                            out3[:, st, hf * 384:(hf + 1) * 384],
                            p3[st * 2 + hf][:],
                            y13[:, st, hf * 384:(hf + 1) * 384],
                        )
                nc.sync.dma_start(t["out"][:], out_sb[:])


def _build(has_inj):
    key = ("nc", has_inj)
    if key in _CACHE:
        return _CACHE[key]
    nc = bacc.Bacc("TRN2", target_bir_lowering=False, debug=False,
                   num_devices=NCORES)
    t = _declare_io(nc, has_inj)
    with tile.TileContext(nc) as tc:
        _build_body(tc, t, has_inj)
    nc.compile()
    _CACHE[key] = nc
    return nc


def _chunk_rows(a, p=128):
    n, m = a.shape
    return np.ascontiguousarray(
        a.reshape(n // p, p, m).transpose(1, 0, 2).reshape(p, -1)
    )


def _f8(a, scale):
    return np.clip(np.asarray(a, np.float32) * scale,
                   -240.0, 240.0).astype(E4NP)


def _prep_in_maps(inputs):
    x = np.asarray(inputs["x"], np.float32)
    Wq = np.asarray(inputs["Wq"], np.float32)
    bq = np.asarray(inputs["bq"], np.float32)
    Wv = np.asarray(inputs["Wv"], np.float32)
    bv = np.asarray(inputs["bv"], np.float32)
    Wo = np.asarray(inputs["Wo"], np.float32)
    bo = np.asarray(inputs["bo"], np.float32)
    ln1_g = np.asarray(inputs["ln1_g"], np.float32)
    ln1_b = np.asarray(inputs["ln1_b"], np.float32)
    W1 = np.asarray(inputs["W1"], np.float32)
    b1 = np.asarray(inputs["b1"], np.float32)
    W2 = np.asarray(inputs["W2"], np.float32)
    b2 = np.asarray(inputs["b2"], np.float32)
    W3 = np.asarray(inputs["W3"], np.float32)
    b3 = np.asarray(inputs["b3"], np.float32)
    ln2_g = np.asarray(inputs["ln2_g"], np.float32)
    ln2_b = np.asarray(inputs["ln2_b"], np.float32)
    mask = np.asarray(inputs["input_mask"])

    # Fold LN affine params into the following projections (exact algebra).
    Wq_eff = Wq * ln1_g[None, :, None]
    bq_eff = bq + np.einsum("e,hed->hd", ln1_b, Wq)
    Wv_eff = Wv * ln1_g[None, :, None]
    bv_eff = bv + np.einsum("e,hed->hd", ln1_b, Wv)
    W1_eff = W1 * ln2_g[:, None]
    b1_eff = b1 + ln2_b @ W1

    w1_p = _chunk_rows(W1_eff).astype(np.float16)
    b1c8 = np.ascontiguousarray((8.0 * b1_eff).reshape(FC, 128).T)
    w2_p = _f8(_chunk_rows(W2), 16.0)
    b2c = np.ascontiguousarray(b2.reshape(FC, 128).T)
    w3_p = _chunk_rows(W3).astype(np.float16)
    b3_bc = np.broadcast_to(b3.astype(np.float32), (128, E)).copy()
    sel = np.zeros((65, 192), np.float16)
    for h_ in range(HPC):
        sel[32 * h_, 64 * h_:64 * (h_ + 1)] = 1.0
    tl = np.arange(128)[:, None]
    slc = np.arange(128)[None, :]
    m_tri = (tl <= slc).astype(np.float16)

    bad_any = {}
    inj_t = {}
    shift_b = {}
    for b_ in range(B):
        bad = (np.cumsum(mask[b_]) == 0)
        assert not bad[128:].any(), "all-masked prefix longer than 128 rows"
        bad_any[b_] = bool(bad[:128].any())
        d = np.zeros((128, 128), np.float16)
        d[np.arange(128), np.arange(128)] = (
            bad[:128].astype(np.float32) * np.float32(2.0 ** -6)
        )
        inj_t[b_] = d
        # calibrate the exp shift: scores/8 <= 0.125*max|q|^2 (Cauchy-Schwarz),
        # keep max pe ~ e^10 so fp16 never overflows.
        xb_ = x[b_]
        mu = xb_.mean(-1, keepdims=True)
        sd = np.sqrt(xb_.var(-1, keepdims=True) + 1e-5)
        ln1 = (xb_ - mu) / sd * ln1_g + ln1_b
        q = np.einsum("se,hed->hsd", ln1, Wq) + bq[:, None, :]
        maxq2 = float((q * q).sum(-1).max())
        shift_b[b_] = min(10.0 - 0.125 * maxq2, 0.0)
    has_inj = any(bad_any.values())

    in_maps = []
    for c in range(NCORES):
        b_, r = c // R, c % R
        hs = [HPC * r + i for i in range(HPC)]

        xb = _chunk_rows(x[b_])
        myrows = np.concatenate(
            [np.arange(512 * J + 128 * r, 512 * J + 128 * r + 128)
             for J in range(4)]
        )
        xmy = _chunk_rows(x[b_, myrows] + bo[None, :])

        Wq_my = np.concatenate([Wq_eff[h] for h in hs], axis=1)
        bq_my = np.concatenate([bq_eff[h] for h in hs])
        wq_p = _chunk_rows(Wq_my).astype(np.float16)
        bqc = np.zeros((128, 2), np.float32)
        bqc[:, 0] = bq_my[:128]
        bqc[:64, 1] = bq_my[128:]

        Wv_aug = np.zeros((E, VW), np.float32)
        bv1 = np.zeros((1, VW), np.float32)
        for i, h in enumerate(hs):
            Wv_aug[:, 65 * i: 65 * i + 64] = Wv_eff[h]
            bv1[0, 65 * i: 65 * i + 64] = bv_eff[h]
            bv1[0, 65 * i + 64] = 1.0
        bv_bc = np.broadcast_to(bv1, (128, VW)).copy()
        wv_p = _chunk_rows(Wv_aug).astype(np.float16)

        # fp16 Wo packed [plane, rows]
        woq = np.zeros((128, 2, E), np.float32)
        woq[:, 0, :] = Wo[hs[0] * D: hs[0] * D + 128]
        woq[:64, 1, :] = Wo[hs[2] * D: hs[2] * D + 64]
        woq = woq.reshape(128, 2 * E).astype(np.float16)

        maskc = np.ascontiguousarray(
            mask[b_].astype(np.float32).reshape(SC, 128).T
        )
        shiftb = np.full((128, 1), shift_b[b_], np.float32)

        in_maps.append({
            "xb": xb, "xmy": xmy,
            "wq": wq_p, "bqc": bqc,
            "wv": wv_p, "bv_bc": bv_bc,
            "maskc": maskc,
            "shiftb": shiftb,
            "woq": woq,
            "m_tri": m_tri,
            "sel": sel,
            "ident": np.eye(128, dtype=np.float16),
            "w1": w1_p, "b1c8": b1c8,
            "w2q": w2_p, "b2c": b2c,
            "w3": w3_p, "b3_bc": b3_bc,
            **({"inj": inj_t[b_]} if has_inj else {}),
        })
    return in_maps, has_inj


def _gather(results):
    y = np.empty((B, S, E), np.float32)
    for c in range(NCORES):
        b_, r = c // R, c % R
        o = results[c]["out"].reshape(128, 4, E).transpose(1, 0, 2).reshape(MYR, E)
        myrows = np.concatenate(
            [np.arange(512 * J + 128 * r, 512 * J + 128 * r + 128)
             for J in range(4)]
        )
        y[b_, myrows] = o
    return y


def run(inputs, **spmd_kwargs):
    in_maps, has_inj = _prep_in_maps(inputs)
    nc = _build(has_inj)
    res = run_bass_kernel_spmd(nc, in_maps, core_ids=list(range(NCORES)),
                               **spmd_kwargs)
    return _gather(res.results), res


def kernel(**inputs) -> np.ndarray:
    y, _ = run(inputs)
    return y


# revision 25
# speedup vs baseline: 1.1258x; 1.1258x over previous
"""Trainium2 Bass kernel for a dense transformer block (B=2,S=2048,E=768,H=12,D=64,F=3072).

Sharding: 8 cores = 2 batch groups x 4 cores. Within a batch group each core
computes attention for 3 of the 12 heads over the full sequence, partial output
projections are combined with a 4-core fp16 ReduceScatter, and each core then
runs the FFN on its 512 rows.

Key structure choices:
- Masked keys are zeroed in V (per-token 0/1 multiply) instead of per-key exp
  biases, so every exp uses one constant bias and adjacent score chunks share
  a single [128,1024] activation (2 PSUM banks).
- Causal masking is done by restricting matmul/exp column ranges per t-chunk;
  only the single 128x128 diagonal block needs an elementwise 0/1 multiply.
- All-masked rows get their diagonal value injected via one extra matmul
  (v_raw^T @ e^-8*diag(bad)) inside the AV accumulation group.
- Softmax reciprocals run as exp(-ln(d)) on the scalar engine over a [3,512]
  tile per row-block (the DVE iterative divide is ~4us per 512-elem row).
- rsqrt in layernorm = exp(-0.5*ln(var+eps)): keeps the scalar engine on one
  activation table set (no exp<->sqrt table thrashing).
- Wo projection and the middle FFN matmul (h1@W2) run in fp8e4m3 DoubleRow
  (2 contraction rows/cycle); h1 is written in fp8 by the ReLU activation.
- FFN1 is split into column groups [128:512] and [0:128] so the bulk of it
  overlaps the last ReduceScatter (row blocks are processed J=3,2,1,0).
"""

import sys

if "/opt/trn_rl_repo" not in sys.path:
    sys.path.insert(0, "/opt/trn_rl_repo")

import numpy as np
import ml_dtypes

import concourse.bacc as bacc
import concourse.mybir as mybir
import concourse.tile as tile
from concourse.bass_utils import run_bass_kernel_spmd

B, S, E, H, D, F = 2, 2048, 768, 12, 64, 3072
NCORES = 8
R = 4          # cores per batch group
HPC = 3        # heads per core
MYR = S // R   # rows per core after reduce-scatter (512)
EC = E // 128  # 6 e-chunks
SC = S // 128  # 16 s-chunks of 128
FC = F // 128  # 24 f-chunks
VW = 256       # padded V width (3 heads x 65 = 195 -> 256)

f32 = mybir.dt.float32
f16 = mybir.dt.float16
f8 = mybir.dt.float8e4
AF = mybir.ActivationFunctionType
ALU = mybir.AluOpType
PM = mybir.MatmulPerfMode

E4NP = ml_dtypes.float8_e4m3
LN16 = float(np.log(16.0))

_CACHE = {}


def _declare_io(nc, has_inj):
    t = {}
    F16_IN = {"wq", "wv", "m_tri", "inj", "ident", "sel", "w1", "w3", "woq"}
    F8_IN = {"w2q"}

    def inp(name, shape):
        dt = f16 if name in F16_IN else (f8 if name in F8_IN else f32)
        t[name] = nc.dram_tensor(name, list(shape), dt, kind="ExternalInput").ap()

    inp("ident", (128, 128))
    inp("m_tri", (128, 128))
    if has_inj:
        inp("inj", (128, 128))
    inp("sel", (65, 192))
    inp("bqc", (128, 2))
    inp("shiftb", (128, 1))
    inp("bv_bc", (128, VW))
    inp("maskc", (128, SC))
    inp("b1c8", (128, FC))
    inp("b2c", (128, FC))
    inp("b3_bc", (128, E))
    inp("xb", (128, SC * E))
    inp("xmy", (128, 4 * E))          # my rows of x[b] with bo pre-added
    inp("wq", (128, EC * 192))
    inp("wv", (128, EC * VW))
    inp("woq", (128, 2 * E))          # fp16: plane0 heads01, plane1 head2+pad
    inp("w1", (128, EC * F))
    inp("w2q", (128, FC * F))         # fp8 x16
    inp("w3", (128, FC * E))
    t["out"] = nc.dram_tensor("out", [128, 4 * E], f32, kind="ExternalOutput").ap()
    return t


def _ln_chunk(nc, pool, x_chunk, out_chunk, out_scale=1.0,
              apply_engine="scalar"):
    """LN a [128, ncols] fp32 chunk into out_chunk, eps=1e-5.

    out = (x - mu) * rsqrt(var+eps) * out_scale, rsqrt via exp(-0.5*ln).
    """
    stats = pool.tile([128, 12], f32, tag="ln_stats")
    nc.vector.bn_stats(stats[:, 0:6], x_chunk[:, 0:384])
    nc.vector.bn_stats(stats[:, 6:12], x_chunk[:, 384:768])
    mv = pool.tile([128, 2], f32, tag="ln_mv")
    nc.vector.bn_aggr(mv[:], stats[:])
    veps = pool.tile([128, 1], f32, tag="ln_veps")
    nc.vector.tensor_scalar_add(veps[:], mv[:, 1:2], 1e-5)
    lnv = pool.tile([128, 1], f32, tag="ln_lnv")
    nc.scalar.activation(lnv[:], veps[:], AF.Ln, bias=0.0, scale=1.0)
    rsig = pool.tile([128, 1], f32, tag="ln_rsig")
    bias = float(np.log(out_scale)) if out_scale != 1.0 else 0.0
    nc.scalar.activation(rsig[:], lnv[:], AF.Exp, bias=bias, scale=-0.5)
    negmurs = pool.tile([128, 1], f32, tag="ln_negmurs")
    nc.vector.scalar_tensor_tensor(
        negmurs[:], mv[:, 0:1], -1.0, rsig[:], ALU.mult, ALU.mult
    )
    if apply_engine == "vector":
        # out = (x + negmurs) * rsig via two per-partition scalars
        nc.vector.tensor_scalar(out_chunk, x_chunk, negmurs[:], rsig[:],
                                ALU.add, ALU.mult)
    else:
        nc.scalar.activation(out_chunk, x_chunk, AF.Identity,
                             bias=negmurs[:], scale=rsig[:])


def _build_body(tc, t, has_inj):
    nc = tc.nc

    with tc.tile_pool(name="constp", bufs=1) as constp, \
         tc.tile_pool(name="lnstat", bufs=4) as lnstat, \
         tc.tile_pool(name="dramp", bufs=1, space="DRAM") as dramp, \
         tc.tile_pool(name="w1pool", bufs=1) as w1pool, \
         tc.tile_pool(name="yp", bufs=1) as yp:
        proj_J = [dramp.tile([MYR, E], f16, name=f"projb_{J}") for J in range(4)]
        rs_J = [dramp.tile([128, E], f16, name=f"rsout_{J}") for J in range(4)]
        warm_in = dramp.tile([4, 64], f16, name="warm_in")
        warm_out = dramp.tile([1, 64], f16, name="warm_out")

        # small constants first so transposes never wait behind x
        ident = constp.tile([128, 128], f16)
        nc.sync.dma_start(ident[:], t["ident"][:])
        warm_sb = constp.tile([4, 64], f16)
        nc.vector.memset(warm_sb[:], 0.0)
        nc.sync.dma_start(warm_in[:, :], warm_sb[:])
        nc.gpsimd.collective_compute(
            "ReduceScatter", ALU.add,
            replica_groups=[[0, 1, 2, 3], [4, 5, 6, 7]],
            ins=[warm_in[:, :].opt()], outs=[warm_out[:, :].opt()],
        )
        m_tri = constp.tile([128, 128], f16)
        nc.sync.dma_start(m_tri[:], t["m_tri"][:])
        if has_inj:
            inj = constp.tile([128, 128], f16)
            nc.sync.dma_start(inj[:], t["inj"][:])
        sel = constp.tile([65, 192], f16)
        nc.sync.dma_start(sel[:], t["sel"][:])
        bqc = constp.tile([128, 2], f32)
        nc.sync.dma_start(bqc[:], t["bqc"][:])
        bv_bc = constp.tile([128, VW], f32)
        nc.sync.dma_start(bv_bc[:], t["bv_bc"][:])
        maskc = constp.tile([128, SC], f32)
        nc.sync.dma_start(maskc[:], t["maskc"][:])
        b1c8 = constp.tile([128, FC], f32)
        nc.sync.dma_start(b1c8[:], t["b1c8"][:])
        b2c = constp.tile([128, FC], f32)
        nc.sync.dma_start(b2c[:], t["b2c"][:])
        b3_bc = constp.tile([128, E], f32)
        nc.sync.dma_start(b3_bc[:], t["b3_bc"][:])
        bm8 = constp.tile([128, 1], f32)
        nc.sync.dma_start(bm8[:], t["shiftb"][:])
        bl16 = constp.tile([128, 1], f32)
        nc.vector.memset(bl16[:], LN16)

        # long-lived FFN-input tiles
        y1 = yp.tile([128, 4 * E], f32)
        y13 = y1[:].rearrange("p (c e) -> p c e", c=4)
        ylnT = yp.tile([128, EC * MYR], f16)
        ylnT3 = ylnT[:].rearrange("p (e s) -> p e s", e=EC)
        xmy_sb = yp.tile([128, 4 * E], f32)
        xmy3 = xmy_sb[:].rearrange("p (c e) -> p c e", c=4)
        h1T = yp.tile([128, FC * MYR], f8)
        h13 = h1T[:].rearrange("p (f s) -> p f s", f=FC)

        with tc.tile_pool(name="aconstp", bufs=1) as aconstp:
            with tc.tile_pool(name="qvp", bufs=1) as qvp:
                # ------------ Phase A: LN1(x_b) + transpose -> ln1T --------
                with tc.tile_pool(name="ln1Tp", bufs=1) as ln1Tp:
                    ln1T = ln1Tp.tile([128, EC * S], f16)
                    ln1T3 = ln1T[:].rearrange("p (e s) -> p e s", e=EC)
                    with tc.tile_pool(name="xp", bufs=1) as xp, \
                         tc.tile_pool(name="lnxp", bufs=3) as lnxp, \
                         tc.tile_pool(name="psA", bufs=4, space="PSUM") as psA:
                        x_sb = xp.tile([128, SC * E], f32)
                        x3 = x_sb[:].rearrange("p (s e) -> p s e", s=SC)
                        xb3 = t["xb"][:].rearrange("p (s e) -> p s e", s=SC)
                        for xc in range(4):
                            nc.sync.dma_start(
                                x3[:, 4 * xc:4 * (xc + 1), :],
                                xb3[:, 4 * xc:4 * (xc + 1), :],
                            )
                        # queue the rest of the inputs behind x
                        nc.sync.dma_start(xmy_sb[:], t["xmy"][:])
                        wq = aconstp.tile([128, EC * 192], f16)
                        nc.sync.dma_start(wq[:], t["wq"][:])
                        wv = aconstp.tile([128, EC * VW], f16)
                        nc.sync.dma_start(wv[:], t["wv"][:])
                        woq = aconstp.tile([128, 2 * E], f16)
                        nc.sync.dma_start(woq[:], t["woq"][:])
                        w1_sb = w1pool.tile([128, EC * F], f16)
                        nc.sync.dma_start(
                            w1_sb[:].rearrange("p (e f) -> p e f", e=EC),
                            t["w1"][:].rearrange("p (e f) -> p e f", e=EC),
                        )
                        for sc in range(SC):
                            lnx = lnxp.tile([128, E], f16, tag="lnx")
                            _ln_chunk(nc, lnstat, x3[:, sc, :], lnx[:],
                                      apply_engine=("vector" if sc % 2 else
                                                    "scalar"))
                            pt = psA.tile([128, EC * 128], f16, tag="pt")
                            for ec in range(EC):
                                nc.tensor.transpose(
                                    pt[:, ec * 128:(ec + 1) * 128],
                                    lnx[:, ec * 128:(ec + 1) * 128], ident[:]
                                )
                            dst = ln1T3[:, :, sc * 128:(sc + 1) * 128]
                            if sc % 2 == 0:
                                nc.vector.tensor_copy(dst, pt[:])
                            else:
                                nc.scalar.copy(dst, pt[:])

                    # ------------ Phase B: Q^T and V projections ----------
                    wq3 = wq[:].rearrange("p (e m) -> p e m", e=EC)
                    wv3 = wv[:].rearrange("p (e m) -> p e m", e=EC)
                    qh = [qvp.tile([128, S], f16, name=f"qh_{i}")
                          for i in range(HPC)]
                    v_sb = qvp.tile([128, SC * VW], f16)
                    v3 = v_sb[:].rearrange("p (s v) -> p s v", s=SC)
                    v0_raw = qvp.tile([128, VW], f16)
                    for i in range(HPC):
                        nc.vector.memset(qh[i][64:128, :], 0.0)
                    with tc.tile_pool(name="psQ", bufs=3, space="PSUM") as psQ:
                        for g in range(2):
                            m = 128 if g == 0 else 64
                            for s4 in range(4):
                                pq = psQ.tile([128, 512], f32, tag="pq")
                                for ec in range(EC):
                                    nc.tensor.matmul(
                                        pq[:m, :],
                                        wq3[:, ec, g * 128:g * 128 + m],
                                        ln1T3[:, ec, s4 * 512:(s4 + 1) * 512],
                                        start=(ec == 0), stop=(ec == EC - 1),
                                    )
                                if g == 0:
                                    nc.vector.tensor_scalar_add(
                                        qh[0][0:64, s4 * 512:(s4 + 1) * 512],
                                        pq[0:64, :], bqc[0:64, 0:1],
                                    )
                                    nc.vector.tensor_scalar_add(
                                        qh[1][0:64, s4 * 512:(s4 + 1) * 512],
                                        pq[64:128, :], bqc[64:128, 0:1],
                                    )
                                else:
                                    nc.vector.tensor_scalar_add(
                                        qh[2][0:64, s4 * 512:(s4 + 1) * 512],
                                        pq[0:64, :], bqc[0:64, 1:2],
                                    )
                        for sc in range(SC):
                            pv = psQ.tile([128, 512], f32, tag="pq")
                            for ec in range(EC):
                                nc.tensor.matmul(
                                    pv[:, :VW],
                                    ln1T3[:, ec, sc * 128:(sc + 1) * 128],
                                    wv3[:, ec, :],
                                    start=(ec == 0), stop=(ec == EC - 1),
                                )
                            if sc == 0:
                                nc.vector.tensor_tensor(
                                    v0_raw[:], pv[:, :VW], bv_bc[:], ALU.add
                                )
                                nc.vector.tensor_scalar_mul(
                                    v3[:, 0, :], v0_raw[:], maskc[:, 0:1]
                                )
                            else:
                                nc.vector.tensor_tensor(
                                    v3[:, sc, :], pv[:, :VW], bv_bc[:], ALU.add
                                )
                                nc.vector.tensor_scalar_mul(
                                    v3[:, sc, :], v3[:, sc, :], maskc[:, sc:sc + 1]
                                )

                # -------- Phase C: attention + Wo + chunked RS -------------
                oTq = qvp.tile([128, 2 * S], f16, name="oTq")
                oTq3 = oTq[:].rearrange("p (k s) -> p k s", k=2)
                nc.vector.memset(oTq3[64:128, 1, :], 0.0)
                woq3 = woq[:].rearrange("p (k e) -> p k e", k=2)
                with tc.tile_pool(name="pexp", bufs=4) as pexp, \
                     tc.tile_pool(name="osml", bufs=8) as osml, \
                     tc.tile_pool(name="projp", bufs=4) as projp, \
                     tc.tile_pool(name="lnyp", bufs=2) as lnyp, \
                     tc.tile_pool(name="psP", bufs=2, space="PSUM") as psP, \
                     tc.tile_pool(name="psO", bufs=2, space="PSUM") as psO, \
                     tc.tile_pool(name="psM", bufs=1, space="PSUM") as psM, \
                     tc.tile_pool(name="psE", bufs=1, space="PSUM") as psE:
                    def make_steps(J, hh, po, av_state):
                        """Pair steps for one head: (emit_scores, emit_avs)."""
                        qT = qh[hh]
                        steps = []
                        for p2 in range(2 * J):
                            def mk_off(p2=p2, hh=hh):
                                box = {}
                                def scores():
                                    pp2 = psP.tile([128, 1024], f32, tag="pp2")
                                    for half in (0, 1):
                                        tci = 2 * p2 + half
                                        nc.tensor.matmul(
                                            pp2[:, half * 512:(half + 1) * 512],
                                            qT[:, tci * 128:(tci + 1) * 128],
                                            qT[:, J * 512:(J + 1) * 512],
                                            start=True, stop=True,
                                        )
                                    pe2 = pexp.tile([128, 1024], f16, tag="pe2")
                                    nc.scalar.activation(pe2[:], pp2[:], AF.Exp,
                                                         bias=bm8[:], scale=0.125)
                                    box["pe2"] = pe2
                                def avs():
                                    pe2 = box["pe2"]
                                    for half in (0, 1):
                                        tci = 2 * p2 + half
                                        _emit_av(po, hh,
                                                 v3[:, tci, 65 * hh:65 * hh + 65],
                                                 pe2[:, half * 512:(half + 1) * 512],
                                                 0, av_state)
                                return scores, avs
                            steps.append(mk_off())
                        for dp in range(2):
                            def mk_diag(dp=dp, hh=hh):
                                k0 = 2 * dp
                                box = {}
                                def scores():
                                    ppd = psP.tile([128, 1024], f32, tag="pp2")
                                    ppd3 = ppd[:].rearrange("p (k s) -> p k s", k=2)
                                    for i in (0, 1):
                                        tci = 4 * J + k0 + i
                                        nc.tensor.matmul(
                                            ppd3[:, i, 128 * k0:512],
                                            qT[:, tci * 128:(tci + 1) * 128],
                                            qT[:, J * 512 + 128 * k0:(J + 1) * 512],
                                            start=True, stop=True,
                                        )
                                    ped = pexp.tile([128, 1024], f16, tag="pe2")
                                    ped3 = ped[:].rearrange("p (k s) -> p k s", k=2)
                                    nc.scalar.activation(
                                        ped3[:, :, 128 * k0:512],
                                        ppd3[:, :, 128 * k0:512],
                                        AF.Exp, bias=bm8[:], scale=0.125,
                                    )
                                    box["ped3"] = ped3
                                def avs():
                                    ped3 = box["ped3"]
                                    for i in (0, 1):
                                        k = k0 + i
                                        tci = 4 * J + k
                                        nc.vector.tensor_mul(
                                            ped3[:, i, 128 * k:128 * (k + 1)],
                                            ped3[:, i, 128 * k:128 * (k + 1)],
                                            m_tri[:],
                                        )
                                        _emit_av(po, hh,
                                                 v3[:, tci, 65 * hh:65 * hh + 65],
                                                 ped3[:, i, 128 * k:512],
                                                 128 * k, av_state)
                                return scores, avs
                            steps.append(mk_diag())
                        return steps

                    def _emit_av(po, hh, vl, pr, lo, st):
                        st["n"] += 1
                        is_last = st["n"] == st["total"]
                        nc.tensor.matmul(
                            po[:65, lo:512], vl, pr,
                            start=(st["n"] == 1),
                            stop=(is_last and not st["inj"]),
                            skip_group_check=True,
                        )
                        if is_last and st["inj"]:
                            nc.tensor.matmul(
                                po[:65, 0:128],
                                v0_raw[:, 65 * hh:65 * hh + 65],
                                inj[:],
                                start=False, stop=True,
                                skip_group_check=True,
                            )

                    def drain_head(po, hh, dn, sb_os):
                        sb_o = osml.tile([64, 512], f16, tag="sb_o")
                        nc.vector.tensor_copy(sb_o[:], po[0:64, :])
                        nc.vector.tensor_copy(
                            dn[32 * hh:32 * hh + 1, :], po[64:65, :]
                        )
                        sb_os.append(sb_o)

                    def make_recip(J, dn):
                        def emit():
                            lnd = osml.tile([65, 512], f32, tag="lnd")
                            nc.scalar.activation(lnd[:], dn[:], AF.Ln,
                                                 bias=0.0, scale=1.0)
                            rcp = osml.tile([65, 512], f16, tag="rcp")
                            nc.scalar.activation(rcp[:], lnd[:], AF.Exp,
                                                 bias=0.0, scale=-1.0)
                            return rcp
                        return emit

                    def make_rest(J, sb_os):
                        def emit(rcp):
                            for hh in range(HPC):
                                pb = psM.tile([128, 512], f32, tag="misc")
                                nc.tensor.matmul(
                                    pb[0:64, :], sel[:, 64 * hh:64 * (hh + 1)],
                                    rcp[:], start=True, stop=True,
                                )
                                sb_b = osml.tile([64, 512], f16, tag="sb_b")
                                nc.vector.tensor_copy(sb_b[:], pb[0:64, :])
                                plane, off = (0, 64 * hh) if hh < 2 else (1, 0)
                                nc.vector.tensor_mul(
                                    oTq3[off:off + 64, plane,
                                         J * 512:(J + 1) * 512],
                                    sb_os[hh][:], sb_b[:],
                                )
                            for sl in range(4):
                                st_ = 4 * J + sl
                                for half in (0, 1):
                                    pw = psM.tile([128, 512], f32, tag="misc")
                                    nc.tensor.matmul(
                                        pw[:, 0:384],
                                        oTq3[:, 0, st_ * 128:(st_ + 1) * 128],
                                        woq3[:, 0, half * 384:(half + 1) * 384],
                                        start=True, stop=False,
                                    )
                                    nc.tensor.matmul(
                                        pw[:, 0:384],
                                        oTq3[0:64, 1, st_ * 128:(st_ + 1) * 128],
                                        woq3[0:64, 1, half * 384:(half + 1) * 384],
                                        start=False, stop=True,
                                    )
                                    prj = projp.tile([128, 384], f16, tag="prj")
                                    if half == 0:
                                        nc.vector.tensor_copy(prj[:], pw[:, 0:384])
                                    else:
                                        nc.scalar.copy(prj[:], pw[:, 0:384])
                                    nc.sync.dma_start(
                                        proj_J[J][sl * 128:(sl + 1) * 128,
                                                  half * 384:(half + 1) * 384],
                                        prj[:],
                                    )
                            nc.gpsimd.collective_compute(
                                "ReduceScatter",
                                ALU.add,
                                replica_groups=[[0, 1, 2, 3], [4, 5, 6, 7]],
                                ins=[proj_J[J][:, :].opt()],
                                outs=[rs_J[J][:, :].opt()],
                            )
                            rs_sb = yp.tile([128, E], f16, tag="rs_sb",
                                            name=f"rs_sb_{J}", bufs=4)
                            nc.gpsimd.dma_start(rs_sb[:], rs_J[J][:, :])
                            return rs_sb
                        return emit

                    def make_post(J, rs_sb):
                        def emit():
                            nc.vector.tensor_add(y13[:, J, :], rs_sb[:],
                                                 xmy3[:, J, :])
                            lny = lnyp.tile([128, E], f16, tag="lny")
                            _ln_chunk(nc, lnstat, y13[:, J, :], lny[:])
                            nc.vector.tensor_tensor(
                                y13[:, J, :], y13[:, J, :], b3_bc[:], ALU.add
                            )
                            pt2 = psE.tile([128, EC * 128], f16, tag="pt2")
                            for ec in range(EC):
                                nc.tensor.transpose(
                                    pt2[:, ec * 128:(ec + 1) * 128],
                                    lny[:, ec * 128:(ec + 1) * 128],
                                    ident[:]
                                )
                            nc.scalar.copy(
                                ylnT3[:, :, J * 128:(J + 1) * 128], pt2[:]
                            )
                        return emit

                    pend_recip = None
                    pend_rest = None
                    posts = []
                    for J in (0, 1, 2, 3):
                        # previous block's reciprocal goes first on the
                        # scalar queue so its Wo never stalls on it
                        rcp_prev = pend_recip() if pend_recip else None
                        sb_os = []
                        dn = osml.tile([65, 512], f16, tag="dn",
                                       name=f"dn_{J}", bufs=2)
                        nc.vector.memset(dn[:], 1.0)
                        total = 4 * J + 4
                        st0 = {"n": 0, "total": total,
                               "inj": has_inj and J == 0}
                        st1 = {"n": 0, "total": total,
                               "inj": has_inj and J == 0}
                        st2 = {"n": 0, "total": total,
                               "inj": has_inj and J == 0}
                        po0 = psO.tile([128, 512], f32, tag="po")
                        po1 = psO.tile([128, 512], f32, tag="po")
                        steps0 = make_steps(J, 0, po0, st0)
                        steps1 = make_steps(J, 1, po1, st1)
                        for s0, s1 in zip(steps0, steps1):
                            s0[0]()
                            s1[0]()
                            s0[1]()
                            s1[1]()
                        drain_head(po0, 0, dn, sb_os)
                        drain_head(po1, 1, dn, sb_os)
                        # previous block's divide/Wo/RS while this one runs
                        if pend_rest:
                            posts.append(make_post(J - 1, pend_rest(rcp_prev)))
                        po2 = psO.tile([128, 512], f32, tag="po")
                        for s2 in make_steps(J, 2, po2, st2):
                            s2[0]()
                            s2[1]()
                        drain_head(po2, 2, dn, sb_os)
                        pend_recip = make_recip(J, dn)
                        pend_rest = make_rest(J, sb_os)
                    rcp_last = pend_recip()
                    posts.append(make_post(3, pend_rest(rcp_last)))
                    for p_ in posts[:3]:
                        p_()

                    def ffn1(lo, hi):
                        w = hi - lo
                        for fc in range(FC):
                            pf_t = psP.tile([128, 1024], f32, tag="pp2")
                            pf = pf_t[:, 0:w]
                            for ec in range(EC):
                                nc.tensor.matmul(
                                    pf,
                                    w1_sb[:, (ec * F) + fc * 128:
                                          (ec * F) + (fc + 1) * 128],
                                    ylnT3[:, ec, lo:hi],
                                    start=(ec == 0), stop=(ec == EC - 1),
                                )
                            nc.scalar.activation(
                                h13[:, fc, lo:hi], pf, AF.Relu,
                                bias=b1c8[:, fc:fc + 1], scale=8.0,
                            )

                    ffn1(0, 384)
                    posts[3]()
                    ffn1(384, 512)

        # ---------------- Phase E: FFN on my 512 rows ----------------------
        with tc.tile_pool(name="ffp", bufs=1) as ffp:
            h2T = ffp.tile([128, FC * MYR], f16)
            h23 = h2T[:].rearrange("p (f s) -> p f s", f=FC)

            # FFN2: h2T = relu((1/128)*(W2^T @ h1T) + b2), fp8 DoubleRow
            w23 = t["w2q"][:].rearrange("p (f g) -> p f g", f=FC)
            with tc.tile_pool(name="w2p", bufs=2) as w2p, \
                 tc.tile_pool(name="psF2", bufs=8, space="PSUM") as psF2:
                for f2b in range(6):
                    w2t = w2p.tile([128, FC * 512], f8, tag="w2t")
                    nc.sync.dma_start(
                        w2t[:].rearrange("p (f g) -> p f g", f=FC),
                        w23[:, :, f2b * 512:(f2b + 1) * 512],
                    )
                    w2t3 = w2t[:].rearrange("p (f g) -> p f g", f=FC)
                    ph2 = [
                        psF2.tile([128, 512], f32, tag="ph2",
                                  name=f"ph2_{f2b}_{k}")
                        for k in range(4)
                    ]
                    for i in range(FC // 2):
                        for k in range(4):
                            nc.tensor.matmul(
                                ph2[k][:],
                                w2t3[:, 2 * i:2 * i + 2,
                                     k * 128:(k + 1) * 128],
                                h13[:, 2 * i:2 * i + 2, :],
                                start=(i == 0), stop=(i == FC // 2 - 1),
                                perf_mode=PM.DoubleRow,
                            )
                    for k in range(4):
                        fc2 = f2b * 4 + k
                        nc.scalar.activation(
                            h23[:, fc2, :], ph2[k][:], AF.Relu,
                            bias=b2c[:, fc2:fc2 + 1], scale=1.0 / 128.0,
                        )

            # FFN3: out = h2 @ W3 + b3 + y1 (fp16)
            w33 = t["w3"][:].rearrange("p (f e) -> p f e", f=FC)
            with tc.tile_pool(name="w3p", bufs=3) as w3p, \
                 tc.tile_pool(name="outp", bufs=1) as outp, \
                 tc.tile_pool(name="psF3", bufs=1, space="PSUM") as psF3:
                out_sb = outp.tile([128, 4 * E], f32)
                out3 = out_sb[:].rearrange("p (c e) -> p c e", c=4)
                p3 = [
                    psF3.tile([128, 384], f32, tag=f"p3_{st}_{hf}",
                              name=f"p3_{st}_{hf}")
                    for st in range(4) for hf in range(2)
                ]
                for fc in range(FC):
                    w3t = w3p.tile([128, E], f16, tag="w3t")
                    nc.sync.dma_start(w3t[:], w33[:, fc, :])
                    for st in range(4):
                        for hf in range(2):
                            nc.tensor.matmul(
                                p3[st * 2 + hf][:],
                                h23[:, fc, st * 128:(st + 1) * 128],
                                w3t[:, hf * 384:(hf + 1) * 384],
                                start=(fc == 0), stop=(fc == FC - 1),
                            )
                for st in range(4):
                    for hf in range(2):
                        nc.vector.tensor_add(
                            out3[:, st, hf * 384:(hf + 1) * 384],
                            p3[st * 2 + hf][:],
                            y13[:, st, hf * 384:(hf + 1) * 384],
                        )
                nc.sync.dma_start(t["out"][:], out_sb[:])


def _patch_act_tables(nc):
    """All scalar activations used here (Ln, Exp, Identity, Relu, Copy) exist
    in the combined natural_log_exp_and_others table set. Strip them from the
    other sets' availability lists so the table-load pass assigns every
    activation to that one set -- otherwise walrus alternates between the
    exp-only and ln-only sets, spending 1.28us per reload on the scalar
    engine (42 reloads = 54us measured)."""
    import types as _types
    from concourse.hw_specs import get_activation_tables
    import concourse.bass as _bass

    def _insert(self):
        has_activation = any(
            isinstance(i, mybir.InstActivation)
            for b in self.main_func.blocks
            for i in b.instructions
        )
        if not has_activation:
            return
        tables = list(get_activation_tables(self.m.arch).items())
        combined = dict(tables)["natural_log_exp_and_others"]
        filtered = [
            (name, funcs if name == "natural_log_exp_and_others"
             else funcs - combined)
            for name, funcs in tables
        ]
        _bass._bass_rust.insert_act_table_loads(self, filtered)

    nc.insert_act_table_loads = _types.MethodType(_insert, nc)


def _build(has_inj):
    key = ("nc", has_inj)
    if key in _CACHE:
        return _CACHE[key]
    nc = bacc.Bacc("TRN2", target_bir_lowering=False, debug=False,
                   num_devices=NCORES)
    _patch_act_tables(nc)
    t = _declare_io(nc, has_inj)
    with tile.TileContext(nc) as tc:
        _build_body(tc, t, has_inj)
    nc.compile()
    _CACHE[key] = nc
    return nc


def _chunk_rows(a, p=128):
    n, m = a.shape
    return np.ascontiguousarray(
        a.reshape(n // p, p, m).transpose(1, 0, 2).reshape(p, -1)
    )


def _f8(a, scale):
    return np.clip(np.asarray(a, np.float32) * scale,
                   -240.0, 240.0).astype(E4NP)


def _prep_in_maps(inputs):
    x = np.asarray(inputs["x"], np.float32)
    Wq = np.asarray(inputs["Wq"], np.float32)
    bq = np.asarray(inputs["bq"], np.float32)
    Wv = np.asarray(inputs["Wv"], np.float32)
    bv = np.asarray(inputs["bv"], np.float32)
    Wo = np.asarray(inputs["Wo"], np.float32)
    bo = np.asarray(inputs["bo"], np.float32)
    ln1_g = np.asarray(inputs["ln1_g"], np.float32)
    ln1_b = np.asarray(inputs["ln1_b"], np.float32)
    W1 = np.asarray(inputs["W1"], np.float32)
    b1 = np.asarray(inputs["b1"], np.float32)
    W2 = np.asarray(inputs["W2"], np.float32)
    b2 = np.asarray(inputs["b2"], np.float32)
    W3 = np.asarray(inputs["W3"], np.float32)
    b3 = np.asarray(inputs["b3"], np.float32)
    ln2_g = np.asarray(inputs["ln2_g"], np.float32)
    ln2_b = np.asarray(inputs["ln2_b"], np.float32)
    mask = np.asarray(inputs["input_mask"])

    # Fold LN affine params into the following projections (exact algebra).
    Wq_eff = Wq * ln1_g[None, :, None]
    bq_eff = bq + np.einsum("e,hed->hd", ln1_b, Wq)
    Wv_eff = Wv * ln1_g[None, :, None]
    bv_eff = bv + np.einsum("e,hed->hd", ln1_b, Wv)
    W1_eff = W1 * ln2_g[:, None]
    b1_eff = b1 + ln2_b @ W1

    w1_p = _chunk_rows(W1_eff).astype(np.float16)
    b1c8 = np.ascontiguousarray((8.0 * b1_eff).reshape(FC, 128).T)
    w2_p = _f8(_chunk_rows(W2), 16.0)
    b2c = np.ascontiguousarray(b2.reshape(FC, 128).T)
    w3_p = _chunk_rows(W3).astype(np.float16)
    b3_bc = np.broadcast_to(b3.astype(np.float32), (128, E)).copy()
    sel = np.zeros((65, 192), np.float16)
    for h_ in range(HPC):
        sel[32 * h_, 64 * h_:64 * (h_ + 1)] = 1.0
    tl = np.arange(128)[:, None]
    slc = np.arange(128)[None, :]
    m_tri = (tl <= slc).astype(np.float16)

    bad_any = {}
    inj_t = {}
    shift_b = {}
    for b_ in range(B):
        bad = (np.cumsum(mask[b_]) == 0)
        assert not bad[128:].any(), "all-masked prefix longer than 128 rows"
        bad_any[b_] = bool(bad[:128].any())
        d = np.zeros((128, 128), np.float16)
        d[np.arange(128), np.arange(128)] = (
            bad[:128].astype(np.float32) * np.float32(2.0 ** -6)
        )
        inj_t[b_] = d
        # calibrate the exp shift: scores/8 <= 0.125*max|q|^2 (Cauchy-Schwarz),
        # keep max pe ~ e^10 so fp16 never overflows.
        xb_ = x[b_]
        mu = xb_.mean(-1, keepdims=True)
        sd = np.sqrt(xb_.var(-1, keepdims=True) + 1e-5)
        ln1 = (xb_ - mu) / sd * ln1_g + ln1_b
        q = np.einsum("se,hed->hsd", ln1, Wq) + bq[:, None, :]
        maxq2 = float((q * q).sum(-1).max())
        shift_b[b_] = min(10.0 - 0.125 * maxq2, 0.0)
    has_inj = any(bad_any.values())

    in_maps = []
    for c in range(NCORES):
        b_, r = c // R, c % R
        hs = [HPC * r + i for i in range(HPC)]

        xb = _chunk_rows(x[b_])
        myrows = np.concatenate(
            [np.arange(512 * J + 128 * r, 512 * J + 128 * r + 128)
             for J in range(4)]
        )
        xmy = _chunk_rows(x[b_, myrows] + bo[None, :])

        Wq_my = np.concatenate([Wq_eff[h] for h in hs], axis=1)
        bq_my = np.concatenate([bq_eff[h] for h in hs])
        wq_p = _chunk_rows(Wq_my).astype(np.float16)
        bqc = np.zeros((128, 2), np.float32)
        bqc[:, 0] = bq_my[:128]
        bqc[:64, 1] = bq_my[128:]

        Wv_aug = np.zeros((E, VW), np.float32)
        bv1 = np.zeros((1, VW), np.float32)
        for i, h in enumerate(hs):
            Wv_aug[:, 65 * i: 65 * i + 64] = Wv_eff[h]
            bv1[0, 65 * i: 65 * i + 64] = bv_eff[h]
            bv1[0, 65 * i + 64] = 1.0
        bv_bc = np.broadcast_to(bv1, (128, VW)).copy()
        wv_p = _chunk_rows(Wv_aug).astype(np.float16)

        # fp16 Wo packed [plane, rows]
        woq = np.zeros((128, 2, E), np.float32)
        woq[:, 0, :] = Wo[hs[0] * D: hs[0] * D + 128]
        woq[:64, 1, :] = Wo[hs[2] * D: hs[2] * D + 64]
        woq = woq.reshape(128, 2 * E).astype(np.float16)

        maskc = np.ascontiguousarray(
            mask[b_].astype(np.float32).reshape(SC, 128).T
        )
        shiftb = np.full((128, 1), shift_b[b_], np.float32)

        in_maps.append({
            "xb": xb, "xmy": xmy,
            "wq": wq_p, "bqc": bqc,
            "wv": wv_p, "bv_bc": bv_bc,
            "maskc": maskc,
            "shiftb": shiftb,
            "woq": woq,
            "m_tri": m_tri,
            "sel": sel,
            "ident": np.eye(128, dtype=np.float16),
            "w1": w1_p, "b1c8": b1c8,
            "w2q": w2_p, "b2c": b2c,
            "w3": w3_p, "b3_bc": b3_bc,
            **({"inj": inj_t[b_]} if has_inj else {}),
        })
    return in_maps, has_inj


def _gather(results):
    y = np.empty((B, S, E), np.float32)
    for c in range(NCORES):
        b_, r = c // R, c % R
        o = results[c]["out"].reshape(128, 4, E).transpose(1, 0, 2).reshape(MYR, E)
        myrows = np.concatenate(
            [np.arange(512 * J + 128 * r, 512 * J + 128 * r + 128)
             for J in range(4)]
        )
        y[b_, myrows] = o
    return y


def run(inputs, **spmd_kwargs):
    in_maps, has_inj = _prep_in_maps(inputs)
    nc = _build(has_inj)
    res = run_bass_kernel_spmd(nc, in_maps, core_ids=list(range(NCORES)),
                               **spmd_kwargs)
    return _gather(res.results), res


def kernel(**inputs) -> np.ndarray:
    y, _ = run(inputs)
    return y
